# revision 16
# baseline (speedup 1.0000x reference)
"""Trainium2 Bass kernel for nn_HGPSLGNN_NotConnected (two-branch GCN + readout).

Strategy (data-parallel over graphs, 16 graphs per core x 8 cores):
  host: build per-graph dense normalized adjacency A (bf16, self-loops folded
        into the diagonal), stack the two branches' features/weights
        block-diagonally so one matmul stream serves both branches.
  device (per core, per graph):
        h2T  = relu(W2bd.T @ relu(W1bd.T @ xT + b1) + b2)      feature-major
        hg   = (h2T_tile).T @ Wgbd                             node-major, bf16
        outT = hg[:, br].T-contracted with A  (8 src-tile matmuls into PSUM)
        relu(outT + b_gcn) ; pooled[:, g] = sum over nodes
  host: gather pooled [B, 2H] per branch, mean = sum/cnt, tiny readout MLP.
"""

import numpy as np
import ml_dtypes

import concourse.bass as bass
import concourse.tile as tile
from concourse import bacc
from concourse import mybir
from concourse import bass_utils

# Problem constants (hardcoded per contract)
B = 128          # graphs
N = 1024         # nodes per graph
NF = 64          # input features
H = 64           # per-branch hidden
P = 128          # partitions
NCORES = 8
GPC = B // NCORES            # graphs per core = 16
NODES_PC = GPC * N           # nodes per core = 16384
S_TILES = N // P             # src tiles per graph = 8

F32 = mybir.dt.float32
BF16 = mybir.dt.bfloat16
BF16_NP = ml_dtypes.bfloat16
_NC = None
LAST_RESULTS = None
LAST_EXEC_S = None


def _build_nc():
    nc = bacc.Bacc("TRN2", target_bir_lowering=False, debug=False,
                   num_devices=NCORES)

    xT = nc.dram_tensor("xT", [P, NODES_PC], F32, kind="ExternalInput").ap()
    Aa = nc.dram_tensor("Aa", [GPC, P, S_TILES, N], BF16,
                        kind="ExternalInput").ap()
    Ab = nc.dram_tensor("Ab", [GPC, P, S_TILES, N], BF16,
                        kind="ExternalInput").ap()
    W1 = nc.dram_tensor("W1", [P, P], F32, kind="ExternalInput").ap()
    W2 = nc.dram_tensor("W2", [P, P], F32, kind="ExternalInput").ap()
    Wg = nc.dram_tensor("Wg", [P, P], F32, kind="ExternalInput").ap()
    b1 = nc.dram_tensor("b1", [P, 1], F32, kind="ExternalInput").ap()
    b2 = nc.dram_tensor("b2", [P, 1], F32, kind="ExternalInput").ap()
    bg = nc.dram_tensor("bg", [P, 1], F32, kind="ExternalInput").ap()
    pooled = nc.dram_tensor("pooled", [P, GPC], F32,
                            kind="ExternalOutput").ap()

    A_dram = [Aa, Ab]

    with tile.TileContext(nc) as tc:
        with (
            tc.tile_pool(name="consts", bufs=1) as consts,
            tc.tile_pool(name="xg", bufs=2) as xpool,
            tc.tile_pool(name="h1c", bufs=2) as h1pool,
            tc.tile_pool(name="h2", bufs=2) as h2pool,
            tc.tile_pool(name="hg", bufs=2) as hgpool,
            tc.tile_pool(name="amat", bufs=4) as apool,
            tc.tile_pool(name="relu", bufs=2) as rpool,
            tc.tile_pool(name="outp", bufs=1) as opool,
            tc.tile_pool(name="ps_pre", bufs=2, space="PSUM") as ps_pre,
            tc.tile_pool(name="ps_hg", bufs=2, space="PSUM") as ps_hg,
            tc.tile_pool(name="ps_mp", bufs=4, space="PSUM") as ps_mp,
        ):
            # Lane-wrap prologue: 16 tiny DMAs cycle all 8 HWDGE sem lanes
            # twice so real DMAs never carry the own-lane recycling wait
            # (walrus DMA wait-slot limit).
            w1t = consts.tile([P, P], F32, tag="w1")
            nc.sync.dma_start(w1t[:], W1)
            w2t = consts.tile([P, P], F32, tag="w2")
            nc.sync.dma_start(w2t[:], W2)
            wgt = consts.tile([P, P], F32, tag="wg")
            nc.sync.dma_start(wgt[:], Wg)
            b1t = consts.tile([P, 1], F32, tag="b1")
            nc.sync.dma_start(b1t[:], b1)
            b2t = consts.tile([P, 1], F32, tag="b2")
            nc.sync.dma_start(b2t[:], b2)
            # gcn biases: per-branch, both at partition base 0
            bgat = consts.tile([H, 1], F32, tag="bga")
            nc.sync.dma_start(bgat[:], bg[0:H, :])
            bgbt = consts.tile([H, 1], F32, tag="bgb")
            nc.sync.dma_start(bgbt[:], bg[H:P, :])

            pooled_a = opool.tile([H, GPC], F32, tag="pa")
            pooled_b = opool.tile([H, GPC], F32, tag="pb")
            pooled_t = [pooled_a, pooled_b]
            bgt = [bgat, bgbt]

            # Warm-up touches: absorb each const tile's DMA-completion wait
            # into a dedicated tiny PE/ACT instruction so no later fused
            # fp32 Matmult carries more than one sync wait (walrus S3_LW
            # has a single wait slot).
            ps_warm = ps_pre.tile([1, 1], F32, tag="ps1")
            for wt in (w1t, w2t, wgt):
                nc.tensor.matmul(ps_warm[:], wt[:, 0:1], wt[:, 0:1],
                                 start=True, stop=True)
            scw = consts.tile([1, 1], F32, tag="scw")
            nc.vector.tensor_copy(out=scw[:], in_=ps_warm[:])
            for i, bt in enumerate((b1t, b2t, bgat, bgbt)):
                sc = consts.tile([bt.shape[0], 1], F32, tag=f"scv{i}")
                nc.vector.tensor_copy(out=sc[:], in_=bt[:])

            for g in range(GPC):
                # ---- pre-MLP, both branches stacked on partitions ----
                xg = xpool.tile([P, N], F32, tag="xg")
                nc.gpsimd.dma_start(xg[:], xT[:, g * N:(g + 1) * N])
                h2 = h2pool.tile([P, N], F32, tag="h2")
                for ch in range(2):
                    sl = slice(ch * 512, (ch + 1) * 512)
                    ps1 = ps_pre.tile([P, 512], F32, tag="ps1")
                    nc.tensor.matmul(ps1[:], w1t[:], xg[:, sl],
                                     start=True, stop=True)
                    h1c = h1pool.tile([P, 512], F32, tag="h1c")
                    nc.vector.tensor_scalar(h1c[:], ps1[:], b1t[:], 0.0,
                                            mybir.AluOpType.add,
                                            mybir.AluOpType.max)
                    ps2 = ps_pre.tile([P, 512], F32, tag="ps1")
                    nc.tensor.matmul(ps2[:], w2t[:], h1c[:],
                                     start=True, stop=True)
                    nc.vector.tensor_scalar(h2[:, sl], ps2[:], b2t[:], 0.0,
                                            mybir.AluOpType.add,
                                            mybir.AluOpType.max)

                # ---- hg node-major (stacked: cols 0:64 = a, 64:128 = b) ----
                hg = hgpool.tile([P, S_TILES, P], BF16, tag="hg")
                for s in range(S_TILES):
                    ps3 = ps_hg.tile([P, P], F32, tag="ps3")
                    nc.tensor.matmul(ps3[:], h2[:, s * P:(s + 1) * P], wgt[:],
                                     start=True, stop=True)
                    nc.vector.tensor_copy(out=hg[:, s, :], in_=ps3[:])

                # ---- message passing + relu + pool, per branch ----
                for br in range(2):
                    At = apool.tile([P, S_TILES, N], BF16, tag="A")
                    nc.gpsimd.dma_start(At[:], A_dram[br][g])
                    relu_t = rpool.tile([H, N], F32, tag="relu")
                    for dh in range(2):
                        mp = ps_mp.tile([H, 512], F32, tag="mp")
                        for s in range(S_TILES):
                            nc.tensor.matmul(
                                mp[:],
                                hg[:, s, br * H:(br + 1) * H],
                                At[:, s, dh * 512:(dh + 1) * 512],
                                start=(s == 0), stop=(s == S_TILES - 1))
                        nc.vector.tensor_scalar(
                            relu_t[:, dh * 512:(dh + 1) * 512], mp[:],
                            bgt[br][:], 0.0,
                            mybir.AluOpType.add, mybir.AluOpType.max)
                    nc.vector.tensor_reduce(
                        pooled_t[br][:, g:g + 1], relu_t[:],
                        axis=mybir.AxisListType.X, op=mybir.AluOpType.add)

            nc.sync.dma_start(pooled[0:H, :], pooled_a[:])
            nc.sync.dma_start(pooled[H:P, :], pooled_b[:])

    nc.compile()
    return nc


def _branch_host(x, edge_index, edge_attr):
    """Per-branch host preprocessing: per-core xT shards and dense bf16 A."""
    src = edge_index[0].astype(np.int64)
    dst = edge_index[1].astype(np.int64)
    ew = edge_attr.astype(np.float64)
    nnodes = x.shape[0]
    deg = np.bincount(dst, weights=ew, minlength=nnodes) + 1.0
    dinv = 1.0 / np.sqrt(deg)
    norm = (dinv[src] * ew * dinv[dst])          # f64 [E]
    g_of_edge = src // N

    A_shards = []
    for c in range(NCORES):
        glo, ghi = c * GPC, (c + 1) * GPC
        m = (g_of_edge >= glo) & (g_of_edge < ghi)
        s_l = src[m] - glo * N
        d_l = dst[m] - glo * N
        g_l = s_l // N
        flat = g_l * (N * N) + (s_l % N) * N + (d_l % N)
        A = np.bincount(flat, weights=norm[m],
                        minlength=GPC * N * N).astype(np.float32)
        A = A.reshape(GPC, N, N)
        idx = np.arange(N)
        dinv_c = dinv[glo * N:ghi * N].reshape(GPC, N)
        A[:, idx, idx] += (dinv_c * dinv_c).astype(np.float32)
        # [g, src, dst] -> [g, 128 part, 8 s_tile, 1024 dst]
        A = A.reshape(GPC, S_TILES, P, N).transpose(0, 2, 1, 3)
        A_shards.append(np.ascontiguousarray(A).astype(BF16_NP))
    return A_shards


def _block_diag(wa, wb):
    out = np.zeros((wa.shape[0] + wb.shape[0], wa.shape[1] + wb.shape[1]),
                   np.float32)
    out[:wa.shape[0], :wa.shape[1]] = wa
    out[wa.shape[0]:, wa.shape[1]:] = wb
    return out


def kernel(x_a, edge_index_a, edge_attr_a, x_a_batch,
           x_b, edge_index_b, edge_attr_b, x_b_batch, linker_size,
           W_pre1_a, b_pre1_a, W_pre2_a, b_pre2_a, W_gcn_a, b_gcn_a,
           W_pre1_b, b_pre1_b, W_pre2_b, b_pre2_b, W_gcn_b, b_gcn_b,
           W_lin1, b_lin1, W_lin2, b_lin2, W_lin3, b_lin3,
           W_cb, b_cb, W_om, b_om, W_th, b_th, W_ph, b_ph):
    x_a = np.asarray(x_a, np.float32)
    x_b = np.asarray(x_b, np.float32)

    Aa_shards = _branch_host(x_a, np.asarray(edge_index_a),
                             np.asarray(edge_attr_a, np.float32))
    Ab_shards = _branch_host(x_b, np.asarray(edge_index_b),
                             np.asarray(edge_attr_b, np.float32))

    W1 = _block_diag(np.asarray(W_pre1_a, np.float32),
                     np.asarray(W_pre1_b, np.float32))
    W2 = _block_diag(np.asarray(W_pre2_a, np.float32),
                     np.asarray(W_pre2_b, np.float32))
    Wg = _block_diag(np.asarray(W_gcn_a, np.float32),
                     np.asarray(W_gcn_b, np.float32))
    b1 = np.concatenate([np.asarray(b_pre1_a, np.float32),
                         np.asarray(b_pre1_b, np.float32)]).reshape(P, 1)
    b2 = np.concatenate([np.asarray(b_pre2_a, np.float32),
                         np.asarray(b_pre2_b, np.float32)]).reshape(P, 1)
    bg = np.concatenate([np.asarray(b_gcn_a, np.float32),
                         np.asarray(b_gcn_b, np.float32)]).reshape(P, 1)

    in_maps = []
    for c in range(NCORES):
        lo, hi = c * NODES_PC, (c + 1) * NODES_PC
        xT = np.ascontiguousarray(
            np.concatenate([x_a[lo:hi].T, x_b[lo:hi].T], axis=0))
        in_maps.append({
            "xT": xT, "Aa": Aa_shards[c], "Ab": Ab_shards[c],
            "W1": W1, "W2": W2, "Wg": Wg, "b1": b1, "b2": b2, "bg": bg,
        })

    global _NC
    if _NC is None:
        _NC = _build_nc()
    import time as _time
    t0 = _time.time()
    res = bass_utils.run_bass_kernel_spmd(_NC, in_maps,
                                          core_ids=list(range(NCORES)))
    global LAST_RESULTS, LAST_EXEC_S
    LAST_RESULTS = res
    LAST_EXEC_S = _time.time() - t0

    s_a = np.zeros((B, H), np.float32)
    s_b = np.zeros((B, H), np.float32)
    for c in range(NCORES):
        pooled = np.asarray(res.results[c]["pooled"], np.float32)
        s_a[c * GPC:(c + 1) * GPC] = pooled[0:H].T
        s_b[c * GPC:(c + 1) * GPC] = pooled[H:P].T

    # ---- host readout (tiny [B, 257] MLP) ----
    cnt_a = np.bincount(np.asarray(x_a_batch), minlength=B)[:, None]
    cnt_b = np.bincount(np.asarray(x_b_batch), minlength=B)[:, None]
    xa = np.concatenate([s_a, s_a / cnt_a.astype(np.float32)], axis=1)
    xb = np.concatenate([s_b, s_b / cnt_b.astype(np.float32)], axis=1)
    x = np.concatenate([xa, xb, np.asarray(linker_size, np.float32)], axis=1)
    x = np.maximum(x @ np.asarray(W_lin1, np.float32)
                   + np.asarray(b_lin1, np.float32), 0.0)
    x = np.maximum(x @ np.asarray(W_lin2, np.float32)
                   + np.asarray(b_lin2, np.float32), 0.0)
    ca = x @ np.asarray(W_lin3, np.float32) + np.asarray(b_lin3, np.float32)
    cb = x @ np.asarray(W_cb, np.float32) + np.asarray(b_cb, np.float32)
    om = x @ np.asarray(W_om, np.float32) + np.asarray(b_om, np.float32)
    th = x @ np.asarray(W_th, np.float32) + np.asarray(b_th, np.float32)
    ph = x @ np.asarray(W_ph, np.float32) + np.asarray(b_ph, np.float32)
    return (ca, cb, om, th, ph)
